# revision 7
# baseline (speedup 1.0000x reference)
"""Trainium2 Bass kernel: batched discrete Kalman filter (B=32, T=128, N=128, M=64).

Distribution: data-parallel over batch, 4 batch elements per core x 8 cores.

Structure exploited:
- P0 is batch-uniform (verified at runtime), so the covariance recursion
  (P, innovation covariance Py, Kalman gain) is identical across the batch:
  each core computes it once (replicated across cores -> no collectives) and
  only the cheap state recursion x is per-batch.  The P outputs are therefore
  written once per core as [T, N, N] and replicated over the batch dim on the
  host during unsharding.
- The observation schedule is read from obs_index on the host at trace time:
  control flow is static.  Between observations, predicted covariances are
  independent jumps  P = F^j Pu (F^j)^T + Q_j  (F^j / Q_j built on-device at
  setup), batched into wide matmuls and kept off the sequential chain.
- inv(Py) (64x64 SPD, lambda_min >= 1 because R = I plus a PSD term) via
  Newton-Schulz iteration on the tensor engine, warm-started from the
  previous observation's inverse (iteration counts validated offline with
  ample margin).
- P outputs accumulate into SBUF group buffers and are flushed with one DMA
  per ~16 timesteps to minimize DGE descriptor overhead.
"""
import numpy as np
from contextlib import ExitStack

B, T, N, M = 32, 128, 128, 64
NCORES = 8
NB = B // NCORES
COLD_ITERS = 10
WARM_SCHED = (4, 4, 3, 2)   # iters for obs 1,2,3,4+ (obs 0 is cold)
GRP_SLOTS = 24               # group buffer capacity (timesteps)
FLUSH_SLOTS = 16             # flush at segment end when this full
MAX_CHUNK = 4                # max jump powers per batched matmul

_prog_cache = {}


def _schedule(obs_index, Tn, K):
    is_obs = np.zeros(Tn, dtype=bool)
    is_obs[np.asarray(obs_index, dtype=np.int64)] = True
    prev = np.concatenate([np.zeros(1, dtype=bool), is_obs[:-1]])
    slot = np.clip(np.cumsum(is_obs.astype(np.int64)) - 1, 0, K - 1)
    return is_obs, prev, slot


def _numpy_fallback(y, x0, P0, obs_index, F, H, Q, R):
    """Reference-exact numpy path, used only if P0 is not batch-uniform."""
    f32 = np.float32
    Bn, K = y.shape[0], y.shape[1]
    is_obs, prev, slot = _schedule(obs_index, T, K)
    x = x0.astype(f32)
    P = P0.astype(f32)
    Ft = F.T.astype(f32)
    Ht = H.T.astype(f32)
    xs = np.zeros((Bn, T, N, 1), f32)
    Ps = np.zeros((Bn, T, N, N), f32)
    for t in range(T):
        if not prev[t]:
            x = np.einsum('ij,bjk->bik', F, x).astype(f32)
            P = (np.einsum('ij,bjk->bik', F, P) @ Ft + Q).astype(f32)
        if is_obs[t]:
            yk = y[:, slot[t]]
            Py = (np.einsum('ij,bjk->bik', H, P) @ Ht + R).astype(f32)
            Kg = (P @ Ht @ np.linalg.inv(Py)).astype(f32)
            x = (x + Kg @ (yk - np.einsum('ij,bjk->bik', H, x))).astype(f32)
            P = (P - Kg @ Py @ np.swapaxes(Kg, -1, -2)).astype(f32)
        xs[:, t] = x
        Ps[:, t] = P
    return xs, Ps


# packed constants layout (columns of the single [128, CW] input tensor)
C_FT = 0          # F^T               [128, 0:128]
C_F = 128         # F                 [128, 128:256]
C_Q = 256         # Q                 [128, 256:384]
C_P0 = 384        # P0 (shared slice) [128, 384:512]
C_HT = 512        # H^T               [128, 512:576]
C_R = 576         # R                 [rows 0:64, 576:640]
C_I64 = 640       # I_64              [rows 0:64, 640:704]
C_X0 = 704        # x0 local          [128, 704:704+NB]
C_Y = 704 + NB    # y local           [rows 0:64, C_Y:C_Y+K*NB]


def _build_program(is_obs, prev, slot, K, maxpow):
    import concourse.tile as tile
    from concourse import bacc, mybir

    f32 = mybir.dt.float32
    AT = mybir.AluOpType
    AF = mybir.ActivationFunctionType
    AX = mybir.AxisListType

    CW = C_Y + K * NB
    nc = bacc.Bacc("TRN2", target_bir_lowering=False, debug=False,
                   enable_asserts=False, num_devices=NCORES)

    consts_d = nc.dram_tensor("consts_in", [N, CW], f32, kind="ExternalInput").ap()
    ps_out = nc.dram_tensor("ps_out", [T, N, N], f32, kind="ExternalOutput").ap()
    xs_out = nc.dram_tensor("xs_out", [N, T * NB], f32, kind="ExternalOutput").ap()

    obs_times = np.flatnonzero(is_obs).tolist()

    with tile.TileContext(nc) as tc:
        with ExitStack() as ctx:
            const = ctx.enter_context(tc.tile_pool(name="const", bufs=1))
            work = ctx.enter_context(tc.tile_pool(name="work", bufs=6))
            grp_pool = ctx.enter_context(tc.tile_pool(name="grp", bufs=3))
            xpool = ctx.enter_context(tc.tile_pool(name="xpool", bufs=4))
            psBig = ctx.enter_context(tc.tile_pool(name="psBig", bufs=3, space="PSUM"))
            psS = ctx.enter_context(tc.tile_pool(name="psS", bufs=3, space="PSUM"))
            psX = ctx.enter_context(tc.tile_pool(name="psX", bufs=2, space="PSUM"))

            CS = const.tile([N, CW], f32)
            nc.sync.dma_start(CS[:], consts_d[:])
            Ft = CS[:, C_FT:C_FT + N]
            F_sb = CS[:, C_F:C_F + N]
            Ht = CS[:, C_HT:C_HT + M]
            R_sb = CS[:64, C_R:C_R + M]
            I64 = CS[:64, C_I64:C_I64 + M]
            P0_sb = CS[:, C_P0:C_P0 + N]
            x0_sb = CS[:, C_X0:C_X0 + NB]

            Fjt = const.tile([N, maxpow * N], f32)   # (F^j)^T at cols j*N (j>=1)
            Qj = const.tile([N, maxpow * N], f32)    # Q_j at cols j*N (j>=1)
            ones = const.tile([M, M], f32)
            xs_sb = const.tile([N, T * NB], f32)
            nc.vector.memset(ones[:], 1.0)
            nc.scalar.copy(Fjt[:, N:2 * N], Ft)
            nc.scalar.copy(Qj[:, N:2 * N], CS[:, C_Q:C_Q + N])
            for j in range(2, maxpow):
                fp = psBig.tile([N, 512], f32, tag="psBig")
                nc.tensor.matmul(fp[:, :N], F_sb, Fjt[:, (j - 1) * N:j * N])
                nc.vector.tensor_copy(Fjt[:, j * N:(j + 1) * N], fp[:, :N])
                aps = psBig.tile([N, 512], f32, tag="psBig")
                nc.tensor.matmul(aps[:, :N], Qj[:, (j - 1) * N:j * N], Ft)
                a_sb = work.tile([N, N], f32, tag="qtmp")
                nc.vector.tensor_copy(a_sb[:], aps[:, :N])
                qp = psBig.tile([N, 512], f32, tag="psBig")
                nc.tensor.matmul(qp[:, :N], Ft, a_sb[:])
                nc.vector.tensor_add(Qj[:, j * N:(j + 1) * N], qp[:, :N],
                                     Qj[:, N:2 * N])

            # ---------- group buffer management ----------
            # P outputs for consecutive timesteps accumulate in an SBUF buffer
            # [N, GRP_SLOTS*N]; one DMA flushes a whole buffer to ps_out.
            gstate = {"tile": None, "t0": 0, "nslots": 0}

            def grp_flush():
                if gstate["tile"] is None or gstate["nslots"] == 0:
                    return
                ns = gstate["nslots"]
                src = gstate["tile"][:, :ns * N].rearrange("p (t n) -> p t n", t=ns)
                dst = ps_out[gstate["t0"]:gstate["t0"] + ns].transpose([1, 0, 2])
                nc.sync.dma_start(dst, src)
                gstate["tile"] = None
                gstate["nslots"] = 0

            def grp_cols(t, nslots):
                """Contiguous [N, nslots*N] slice for timesteps t..t+nslots-1."""
                if gstate["tile"] is not None and \
                        gstate["nslots"] + nslots > GRP_SLOTS:
                    grp_flush()
                if gstate["tile"] is None:
                    gstate["tile"] = grp_pool.tile([N, GRP_SLOTS * N], f32, tag="grp", name="grp")
                    gstate["t0"] = t
                    gstate["nslots"] = 0
                assert gstate["t0"] + gstate["nslots"] == t, "non-contiguous fill"
                c0 = gstate["nslots"] * N
                sl = gstate["tile"][:, c0:c0 + nslots * N]
                gstate["nslots"] += nslots
                return sl

            # ---------- jump emission ----------
            def emit_jumps(post_P, post_x, s_list, t_list, prior_s):
                """Output jumps P_t = F^s Pu F^sT + Q_s for s in s_list (written
                to group slots for t_list), plus optional prior jump prior_s
                (returned as (Pp_sbuf_ap, xp_psum_tile))."""
                all_s = list(s_list) + ([prior_s] if prior_s else [])
                prior_P = None
                i = 0
                while i < len(all_s):
                    chunk = all_s[i:i + MAX_CHUNK]
                    s0 = chunk[0]
                    ncol = len(chunk) * N
                    g_ps = psBig.tile([N, 512], f32, tag="psBig")
                    nc.tensor.matmul(g_ps[:, :ncol], post_P,
                                     Fjt[:, s0 * N:(s0 + len(chunk)) * N])
                    g_sb = work.tile([N, 512], f32, tag="gsb")
                    nc.scalar.copy(g_sb[:, :ncol], g_ps[:, :ncol])
                    out_chunk = [s for s in chunk if prior_s is None or s != prior_s]
                    if out_chunk:
                        so = out_chunk[0]
                        oncol = len(out_chunk) * N
                        pp_ps = psBig.tile([N, 512], f32, tag="psBig")
                        for k, s in enumerate(out_chunk):
                            nc.tensor.matmul(
                                pp_ps[:, k * N:(k + 1) * N],
                                Fjt[:, s * N:(s + 1) * N],
                                g_sb[:, (s - s0) * N:(s - s0 + 1) * N])
                        dst = grp_cols(t_list[so - s_list[0]], len(out_chunk))
                        nc.vector.tensor_add(dst, pp_ps[:, :oncol],
                                             Qj[:, so * N:(so + len(out_chunk)) * N])
                    if prior_s is not None and prior_s in chunk:
                        ppp = psBig.tile([N, 512], f32, tag="psBig")
                        nc.tensor.matmul(ppp[:, :N],
                                         Fjt[:, prior_s * N:(prior_s + 1) * N],
                                         g_sb[:, (prior_s - s0) * N:(prior_s - s0 + 1) * N])
                        pp_sb = work.tile([N, N], f32, tag="ppsb")
                        nc.vector.tensor_add(pp_sb[:], ppp[:, :N],
                                             Qj[:, prior_s * N:(prior_s + 1) * N])
                        prior_P = pp_sb[:]
                    i += len(chunk)
                # x jumps, batched (chunks of <=8)
                i = 0
                while i < len(s_list):
                    xchunk = s_list[i:i + 8]
                    xall = psX.tile([N, 8 * NB], f32, tag="psX")
                    for k, s in enumerate(xchunk):
                        nc.tensor.matmul(xall[:, k * NB:(k + 1) * NB],
                                         Fjt[:, s * N:(s + 1) * N], post_x)
                    t0 = t_list[i]
                    nc.vector.tensor_copy(
                        xs_sb[:, t0 * NB:(t0 + len(xchunk)) * NB],
                        xall[:, :len(xchunk) * NB])
                    i += len(xchunk)
                xp_ps = None
                if prior_s is not None:
                    xp_ps = psX.tile([N, 8 * NB], f32, tag="psX")
                    nc.tensor.matmul(xp_ps[:, :NB], Fjt[:, prior_s * N:(prior_s + 1) * N],
                                     post_x)
                return prior_P, xp_ps

            # ---------- main loop over observation segments ----------
            post_P = P0_sb
            post_x = x0_sb
            post_t = -1
            X_ap = None
            n_obs_done = 0

            for o in obs_times + [None]:
                if o is None:
                    # tail run after last posterior
                    t_copy = post_t + 1
                    if post_t >= 0 and t_copy < T:
                        nc.scalar.copy(grp_cols(t_copy, 1), post_P)
                        nc.scalar.copy(xs_sb[:, t_copy * NB:(t_copy + 1) * NB],
                                       post_x)
                        s_list = list(range(1, T - t_copy))
                        t_list = [t_copy + s for s in s_list]
                    else:
                        s_list = list(range(1, T + 1))
                        t_list = list(range(0, T))
                    if s_list:
                        emit_jumps(post_P, post_x, s_list, t_list, None)
                    grp_flush()
                    break

                g = o - post_t
                if post_t >= 0:
                    if g >= 2:
                        nc.scalar.copy(grp_cols(post_t + 1, 1), post_P)
                        nc.scalar.copy(xs_sb[:, (post_t + 1) * NB:(post_t + 2) * NB],
                                       post_x)
                    s_list = list(range(1, g - 1))
                    t_list = [post_t + 1 + s for s in s_list]
                    prior_s = g - 1 if g >= 2 else None
                else:
                    s_list = list(range(1, o + 1))
                    t_list = list(range(0, o))
                    prior_s = o + 1

                if prior_s is not None:
                    Pp_ap, xp_ps = emit_jumps(post_P, post_x, s_list, t_list, prior_s)
                    xp_sb = work.tile([N, NB], f32, tag="xpsb")
                    nc.vector.tensor_copy(xp_sb[:], xp_ps[:, :NB])
                    xp_ap = xp_sb[:]
                else:
                    if s_list:
                        emit_jumps(post_P, post_x, s_list, t_list, None)
                    Pp_ap = post_P
                    xp_ap = post_x

                # ---------- measurement update at time o ----------
                sl_ = int(slot[o])
                c_ps = psBig.tile([N, 512], f32, tag="psBig")
                nc.tensor.matmul(c_ps[:, :M], Pp_ap, Ht)        # C = Pp Ht
                c_sb = work.tile([N, M], f32, tag="csb")
                nc.vector.tensor_copy(c_sb[:], c_ps[:, :M])
                ct_ps = psS.tile([M, N], f32, tag="psS")
                nc.tensor.matmul(ct_ps[:], Ht, Pp_ap)           # Ct = H Pp
                ct_sb = work.tile([M, N], f32, tag="ctsb")
                nc.vector.tensor_copy(ct_sb[:], ct_ps[:])
                py_ps = psS.tile([M, N], f32, tag="psS")
                nc.tensor.matmul(py_ps[:, :M], Ht, c_sb[:])     # Py = H C (+R)
                py_sb = work.tile([M, M], f32, tag="pysb")
                nc.vector.tensor_add(py_sb[:], py_ps[:, :M], R_sb)

                if X_ap is None:
                    sq = work.tile([M, M], f32, tag="sq")
                    nc.vector.tensor_mul(sq[:], py_sb[:], py_sb[:])
                    rs = work.tile([M, 1], f32, tag="rs")
                    nc.vector.reduce_sum(rs[:], sq[:], axis=AX.X)
                    fro_ps = psS.tile([M, N], f32, tag="psS")
                    nc.tensor.matmul(fro_ps[:, :1], ones[:], rs[:])
                    sroot = work.tile([M, 1], f32, tag="sroot")
                    nc.scalar.activation(sroot[:], fro_ps[:, :1], AF.Sqrt)
                    u = work.tile([M, 1], f32, tag="u")
                    nc.vector.tensor_scalar(u[:], sroot[:], 0.5, 0.5, AT.mult, AT.add)
                    c0 = work.tile([M, 1], f32, tag="c0")
                    nc.vector.reciprocal(c0[:], u[:])
                    X0 = xpool.tile([M, M], f32, tag="X")
                    nc.vector.tensor_scalar(X0[:], I64, c0[:], None, AT.mult)
                    X_ap = X0[:]
                    iters = COLD_ITERS
                else:
                    iters = WARM_SCHED[min(n_obs_done - 1, len(WARM_SCHED) - 1)]

                for _ in range(iters):
                    z_ps = psS.tile([M, N], f32, tag="psS")
                    nc.tensor.matmul(z_ps[:, :M], py_sb[:], X_ap)   # Py X
                    e_sb = work.tile([M, M], f32, tag="esb")
                    nc.vector.tensor_sub(e_sb[:], I64, z_ps[:, :M])
                    w_ps = psS.tile([M, N], f32, tag="psS")
                    nc.tensor.matmul(w_ps[:, :M], X_ap, e_sb[:])    # X (I - Py X)
                    xn = xpool.tile([M, M], f32, tag="X")
                    nc.vector.tensor_add(xn[:], X_ap, w_ps[:, :M])
                    X_ap = xn[:]

                kgt_ps = psS.tile([M, N], f32, tag="psS")
                nc.tensor.matmul(kgt_ps[:], X_ap, ct_sb[:])     # Kg^T = X H Pp
                kgt_sb = work.tile([M, N], f32, tag="kgtsb")
                nc.vector.tensor_copy(kgt_sb[:], kgt_ps[:])

                hxp_ps = psS.tile([M, N], f32, tag="psS")
                nc.tensor.matmul(hxp_ps[:, :NB], Ht, xp_ap)     # H xp
                innov = work.tile([M, NB], f32, tag="innov")
                nc.vector.tensor_sub(innov[:],
                                     CS[:64, C_Y + sl_ * NB:C_Y + (sl_ + 1) * NB],
                                     hxp_ps[:, :NB])
                dx_ps = psX.tile([N, 8 * NB], f32, tag="psX")
                nc.tensor.matmul(dx_ps[:, :NB], kgt_sb[:], innov[:])  # Kg innov
                nc.vector.tensor_add(xs_sb[:, o * NB:(o + 1) * NB], xp_ap, dx_ps[:, :NB])
                post_x = xs_sb[:, o * NB:(o + 1) * NB]

                kc_ps = psBig.tile([N, 512], f32, tag="psBig")
                nc.tensor.matmul(kc_ps[:, :N], ct_sb[:], kgt_sb[:])  # C X Ct
                pu_slot = grp_cols(o, 1)
                nc.vector.tensor_sub(pu_slot, Pp_ap, kc_ps[:, :N])
                post_P = pu_slot
                post_t = o
                n_obs_done += 1
                if gstate["nslots"] >= FLUSH_SLOTS:
                    grp_flush()

            nc.sync.dma_start(xs_out[:], xs_sb[:])

    nc.compile()
    return nc


def kernel(t, y, x0, P0, obs_index, F, H, Q, R):
    y = np.ascontiguousarray(np.asarray(y, dtype=np.float32))
    x0 = np.ascontiguousarray(np.asarray(x0, dtype=np.float32))
    P0 = np.ascontiguousarray(np.asarray(P0, dtype=np.float32))
    obs_index = np.asarray(obs_index)
    F = np.ascontiguousarray(np.asarray(F, dtype=np.float32))
    H = np.ascontiguousarray(np.asarray(H, dtype=np.float32))
    Q = np.ascontiguousarray(np.asarray(Q, dtype=np.float32))
    R = np.ascontiguousarray(np.asarray(R, dtype=np.float32))

    assert y.shape[0] == B and F.shape == (N, N) and H.shape == (M, N)
    K = y.shape[1]

    if not np.all(P0 == P0[0:1]):
        return _numpy_fallback(y, x0, P0, obs_index, F, H, Q, R)

    is_obs, prev, slot = _schedule(obs_index, T, K)

    steps = 0
    maxpow = 3
    for ti in range(T):
        if not prev[ti]:
            steps += 1
        maxpow = max(maxpow, steps + 1)
        if is_obs[ti]:
            steps = 0

    key = (tuple(np.flatnonzero(is_obs).tolist()), K, maxpow)
    if key not in _prog_cache:
        _prog_cache[key] = _build_program(is_obs, prev, slot, K, maxpow)
    nc = _prog_cache[key]

    from concourse.bass_utils import run_bass_kernel_spmd

    CW = C_Y + K * NB
    in_maps = []
    for c in range(NCORES):
        consts = np.zeros((N, CW), np.float32)
        consts[:, C_FT:C_FT + N] = F.T
        consts[:, C_F:C_F + N] = F
        consts[:, C_Q:C_Q + N] = Q
        consts[:, C_P0:C_P0 + N] = P0[0]
        consts[:, C_HT:C_HT + M] = H.T
        consts[:64, C_R:C_R + M] = R
        consts[:64, C_I64:C_I64 + M] = np.eye(M, dtype=np.float32)
        consts[:, C_X0:C_X0 + NB] = x0[c * NB:(c + 1) * NB, :, 0].T
        consts[:64, C_Y:C_Y + K * NB] = (
            y[c * NB:(c + 1) * NB, :, :, 0].transpose(2, 1, 0).reshape(M, K * NB))
        in_maps.append({"consts_in": consts})

    res = run_bass_kernel_spmd(nc, in_maps, core_ids=list(range(NCORES)))

    xs_full = np.zeros((B, T, N, 1), np.float32)
    Ps_full = np.zeros((B, T, N, N), np.float32)
    for c in range(NCORES):
        xs_c = res.results[c]["xs_out"].reshape(N, T, NB)
        xs_full[c * NB:(c + 1) * NB, :, :, 0] = xs_c.transpose(2, 1, 0)
        Ps_full[c * NB:(c + 1) * NB] = res.results[c]["ps_out"][None]
    return xs_full, Ps_full
